# revision 9
# baseline (speedup 1.0000x reference)
"""Trainium2 Bass kernel: row-wise argmax over the vocab axis.

Problem: output = argmax(softmax(x, axis=2), axis=2)[..., None].astype(f32)
for x of shape (16, 512, 32000) f32. Softmax is monotone, so this is a plain
argmax over the last axis.

Sharding: data-parallel over the batch axis — core c handles batches
[2c, 2c+2), i.e. a (1024, 32000) f32 slab per core (131 MB, streamed once).

Per-core algorithm (memory-bound; one DVE pass over the data):
  1. Stream each 128-row tile as two 16000-wide chunks (64 KB contiguous per
     partition per DMA; triple-buffered so the stream has a chunk of
     elasticity against DVE hiccups); tensor_reduce(max) over 128-wide
     blocks -> per-row block maxima [128, 250].
  2. vector.max + max_index over the 250 block maxima -> top-1 value and
     winning block id per row.
  3. Indirect-DMA gather of each row's winning 128-wide block from HBM
     (gpsimd SWDGE, one [128,1]-offset gather per tile — the only indirect
     shape the HW ucode accepts; multi-index offset APs wedge the device).
  4. max_index over the gathered block -> in-block offset.
  5. final index = block_id * 128 + offset, cast to f32 (exact: < 2^24),
     DMA out on the scalar-engine ring.
Steps 4-5 are software-pipelined one tile behind steps 1-3 (and pinned
there with an ordering dep) so the in-order Vector engine never stalls on
the in-flight gather DMA mid-stream.

Measured solo per-core steady state ~310-322 us vs the ~288-298 us
pure-streaming floor (131 MB @ ~440 GB/s solo); with all 8 cores streaming
concurrently the four HBM stacks saturate (~2.9 TB/s chip-wide) and the
per-core steady state is ~360-375 us.
"""

import numpy as np

P = 128          # SBUF partitions / rows per tile
V = 32000        # vocab (reduced axis)
B = 128          # stage-1 block width (gather granularity)
CHUNK = 16000    # free-dim chunk per DMA/reduce (64 KB/partition segments)
BUFS = 3         # chunk buffering depth (3 x 64 KB per partition)
N_CORES = 8
ROWS_PER_CORE = 16 * 512 // N_CORES  # 1024

_cache = {}


def _build(rows, repeat=1, chunk=CHUNK, bufs=BUFS, b=B, alternate_dma=False,
           defer_tail=True, taper_last=None, defer_depth=1):
    import concourse.bass as bass
    import concourse.bacc as bacc
    import concourse.mybir as mybir
    from concourse.tile import TileContext

    f32 = mybir.dt.float32
    i32 = mybir.dt.int32
    u32 = mybir.dt.uint32

    nch = V // chunk
    cb = chunk // b     # blocks per chunk
    nb = V // b         # blocks per row
    assert chunk * nch == V and b * cb == chunk and b * nb == V

    nc = bacc.Bacc(trn_type="TRN2", debug=False)
    x = nc.dram_tensor("x", [rows, V], f32, kind="ExternalInput")
    y = nc.dram_tensor("y", [rows, 1], f32, kind="ExternalOutput")
    x_ap = x.ap()
    x_blocks = x_ap.rearrange("r (n b) -> (r n) b", b=b)  # [rows*nb, b]
    n_tiles = rows // P

    with TileContext(nc) as tc:
        with (
            tc.tile_pool(name="data", bufs=bufs) as dpool,
            tc.tile_pool(name="small", bufs=3) as spool,
            tc.tile_pool(name="cst", bufs=1) as cpool,
        ):
            # rowbase[p, 0] = p * nb  (block-row base within a tile's view)
            rowbase = cpool.tile([P, 1], i32)
            nc.gpsimd.iota(rowbase[:], [[1, 1]], base=0, channel_multiplier=nb)

            from concourse.tile import add_dep_helper

            def tail(t, top8, blk8, gath, after=None):
                """Consume the gathered winning block -> final f32 index."""
                inb8 = spool.tile([P, 8], u32, tag="inb8")
                mi = nc.vector.max_index(
                    out=inb8[:], in_max=top8[:], in_values=gath[:]
                )
                if after is not None:
                    # pin the gather-consuming op after the newest reduce so
                    # the scheduler cannot hoist it into the streaming stretch
                    add_dep_helper(mi.ins, after.ins, sync=False,
                                   reason="tail after current tile reduces")
                # final = block_id * b + in_block_offset, in f32 (exact: < 2^24)
                fblk = spool.tile([P, 1], f32, tag="fblk")
                finb = spool.tile([P, 1], f32, tag="finb")
                nc.vector.tensor_copy(out=fblk[:], in_=blk8[:, 0:1])
                nc.vector.tensor_copy(out=finb[:], in_=inb8[:, 0:1])
                res = spool.tile([P, 1], f32, tag="res")
                nc.vector.scalar_tensor_tensor(
                    out=res[:],
                    in0=fblk[:],
                    scalar=float(b),
                    in1=finb[:],
                    op0=mybir.AluOpType.mult,
                    op1=mybir.AluOpType.add,
                )
                # scalar-engine HWDGE ring: a waiting store never head-blocks
                # the SP ring that feeds the streaming chunk loads
                nc.scalar.dma_start(out=y.ap()[t * P:(t + 1) * P, :], in_=res[:])

            # Chunk widths per tile: uniform big chunks, except the LAST tile
            # may split its final chunk so the last reduce (which serializes
            # after the final byte of the stream) is short. Pieces stay large
            # (multiples of b, >= ~8 KB/partition) — tiny DMAs cost more in
            # issue latency than the reduce they save.
            uniform = [chunk] * nch
            if taper_last:
                pieces = [(p // b) * b for p in taper_last]
                assert sum(pieces) == chunk and all(p > 0 for p in pieces), pieces
                tapered = uniform[:-1] + pieces
            else:
                tapered = uniform
            assert sum(tapered) == V and all(c % b == 0 for c in tapered)

            pending = []
            for rep in range(repeat):
                for t in range(n_tiles):
                    blockmax = spool.tile([P, nb], f32, tag="blockmax")
                    last_reduce = None
                    col = 0
                    for w in (tapered if t == n_tiles - 1 else uniform):
                        ch = dpool.tile([P, chunk], f32, tag="chunk")
                        dma_eng = nc.scalar if (alternate_dma and col % (2 * chunk)) else nc.sync
                        dma_eng.dma_start(
                            out=ch[:, :w],
                            in_=x_ap[t * P:(t + 1) * P, col:col + w],
                        )
                        last_reduce = nc.vector.tensor_reduce(
                            out=blockmax[:, col // b:(col + w) // b],
                            in_=ch[:, :w].rearrange("p (n b) -> p n b", b=b),
                            axis=mybir.AxisListType.X,
                            op=mybir.AluOpType.max,
                        )
                        col += w

                    top8 = spool.tile([P, 8], f32, tag="top8")
                    blk8 = spool.tile([P, 8], u32, tag="blk8")
                    gath = spool.tile([P, b], f32, tag="gath")
                    gidx = spool.tile([P, 1], i32, tag="gidx")
                    nc.vector.max(out=top8[:], in_=blockmax[:])
                    nc.vector.max_index(
                        out=blk8[:], in_max=top8[:], in_values=blockmax[:]
                    )
                    # winning block, as an index into x_blocks local to this tile
                    nc.vector.tensor_tensor(
                        out=gidx[:],
                        in0=rowbase[:],
                        in1=blk8[:, 0:1].bitcast(i32),
                        op=mybir.AluOpType.add,
                    )
                    nc.gpsimd.indirect_dma_start(
                        out=gath[:],
                        out_offset=None,
                        in_=x_blocks,
                        in_offset=bass.IndirectOffsetOnAxis(ap=gidx[:, 0:1], axis=0),
                        element_offset=t * P * V,
                    )
                    if defer_tail:
                        # N-tile software pipeline: consume tile t-N's gather
                        # while tile t+1 streams, so DVE never stalls on the
                        # in-flight gather DMA
                        pending.append((t, top8, blk8, gath))
                        if len(pending) > defer_depth:
                            tail(*pending.pop(0), after=last_reduce)
                    else:
                        tail(t, top8, blk8, gath)

                for args in pending:
                    tail(*args)
                pending = []
    nc.compile()
    return nc


def get_nc(rows=ROWS_PER_CORE, repeat=1):
    key = (rows, repeat)
    if key not in _cache:
        _cache[key] = _build(rows, repeat)
    return _cache[key]


def kernel(output: np.ndarray) -> np.ndarray:
    """Full-input entry point: (16, 512, 32000) f32 -> (16, 512, 1) f32."""
    from concourse.bass_utils import run_bass_kernel_spmd

    n, d, v = output.shape
    assert (n, d, v) == (16, 512, V), (n, d, v)
    x = np.ascontiguousarray(output, dtype=np.float32).reshape(
        N_CORES, ROWS_PER_CORE, V
    )
    nc = get_nc(ROWS_PER_CORE)
    in_maps = [{"x": x[c]} for c in range(N_CORES)]
    res = run_bass_kernel_spmd(nc, in_maps, core_ids=list(range(N_CORES)))
    out = np.stack([res.results[c]["y"] for c in range(N_CORES)], axis=0)
    return out.reshape(n, d, 1).astype(np.float32)

